# revision 1
# baseline (speedup 1.0000x reference)
"""CTC loss (tf.keras ctc_batch_cost semantics) on 8 Trainium2 NeuronCores.

Sharding: data-parallel over batch -- each of the 8 cores handles 32
examples end-to-end (the CTC DP is independent per example); the host
concatenates the per-core [32, 1] losses.

Math: the CTC forward runs in *linear* probability space with a constant
per-step boost  p~ = K * (y_pred + eps), K = e^0.15.  Every path through
the T=512 trellis picks up exactly T boost factors, so
loss = -(ln(alpha_T[S-1] + alpha_T[S-2]) - T*ln K).  K is tuned so the
whole trellis stays inside fp32 range on these inputs (peak ~5e34);
values that underflow to zero correspond to paths ~e^-90 below the
dominant ones -- numerically irrelevant, the same role the -1e30 "NEG"
plays in the reference's log-space DP.

The recurrence splits into even (blank) and odd (label) lanes:
    E[j,t] = pb[t] * (E[j,t-1] + O[j-1,t-1])                       (s = 2j)
    O[j,t] = pl[j,t] * (O[j,t-1] + E[j,t-1] + sk[j]*O[j-1,t-1])    (s = 2j+1)
Each lane is a first-order linear recurrence along t, which maps to ONE
DVE `tensor_tensor_scan` instruction (state = d0*state + d1) covering all
512 time steps -- the sequential dimension collapses from T=512 elementwise
steps (the reference's scan) to 65 lane sweeps of <=5 wide vector ops.
The DP runs in fp32; end-to-end error vs the fp32 log-space reference is
~1.5e-4 (from the fp16 rounding of p~, verified in sim and on HW).

Gather: the DP needs p~ at the per-example ext-label columns in a
[b(partitions), j, t] layout.  Labels are constant over t, so this is a
column gather of each y_pred[b]: y_pred is cast to fp16 inline by a SWDGE
DMA, xbar-DMA-transposed into two [c_half, (b, t)] SBUF tiles, and each
example's 65 needed rows are pulled out by a small one-hot matmul on the
(otherwise idle) PE -- lhsT = host-built one-hot [128c, 65] with value K
(folding the boost into the exact 0/K matmul), rhs = the transposed
[128c, 512t] slice, accumulated over the two c-halves into PSUM, then
copied to SBUF as fp16 with the +K*eps folded into the copy bias.
(An indirect-DMA row gather would be more direct, but SWDGE dynamic
DMA is not executable on this runtime -- it kills the exec unit.)
"""
import numpy as np

import concourse.bass as bass
import concourse.bacc as bacc
import concourse.tile as tile
from concourse import mybir
from concourse.bass_utils import run_bass_kernel_spmd

B, T, C, L = 256, 512, 256, 64
NCORES = 8
BC = B // NCORES
NL = L + 1
EPS = 1e-7
CBOOST = 0.15
KF = float(np.float16(np.exp(CBOOST)))     # fp16-representable boost
CB_EFF = float(np.log(KF))

F32 = mybir.dt.float32
F16 = mybir.dt.float16

NCH = 8
ROWS = BC * T              # 16384
CH_ROWS = ROWS // NCH      # 2048


def _emit(nc, tc, yp, oh, sks, loss):
    dramp = tc.alloc_tile_pool(name="dram", bufs=1, space="DRAM")
    ybf_chunks = [
        dramp.tile([CH_ROWS, C], F16, name=f"ybf{ch}") for ch in range(NCH)
    ]
    yp_ap = yp[:]

    with tc.tile_pool(name="dp", bufs=1) as dp:
        skt = dp.tile([BC, L], F32, name="skt")
        nc.sync.dma_start(out=skt[:], in_=sks[:])
        ohsb = dp.tile([128, BC * 2 * NL], F16, name="ohsb")
        nc.sync.dma_start(out=ohsb[:], in_=oh[:])
        pl = dp.tile([BC, NL * T], F16, name="pl")

        with tc.tile_pool(name="stage", bufs=4) as sp:
            for ch in range(NCH):
                r0 = ch * CH_ROWS
                ycast = sp.tile([128, CH_ROWS * C // 128], F16, tag="ycast")
                nc.gpsimd.dma_start(
                    out=ycast[:],
                    in_=yp_ap[r0:r0 + CH_ROWS, :].rearrange(
                        "(p r) c -> p (r c)", p=128),
                )
                nc.sync.dma_start(
                    out=ybf_chunks[ch][:].rearrange(
                        "(p r) c -> p (r c)", p=128),
                    in_=ycast[:],
                )

        with (
            tc.tile_pool(name="xp", bufs=1) as xp,
            tc.tile_pool(name="ps", bufs=8, space="PSUM") as ps,
            tc.tile_pool(name="cp", bufs=8) as cp,
        ):
            yth = [xp.tile([128, ROWS], F16, name=f"yth{h}") for h in range(2)]
            for ch in range(NCH):
                for half in range(2):
                    eng = nc.sync if half == 0 else nc.scalar
                    eng.dma_start_transpose(
                        out=yth[half][:, ch * CH_ROWS:(ch + 1) * CH_ROWS],
                        in_=ybf_chunks[ch][:, half * 128:(half + 1) * 128],
                    )

            # per-example gather: PSUM[j,t] = sum_c oh[c, (b,h,j)] * yth[h][c, (b,t)]
            for b in range(BC):
                pst = ps.tile([NL, T], F32, tag="pst")
                for half in range(2):
                    nc.tensor.matmul(
                        pst[:],
                        lhsT=ohsb[:, (b * 2 + half) * NL:(b * 2 + half + 1) * NL],
                        rhs=yth[half][:, b * T:(b + 1) * T],
                        start=(half == 0), stop=(half == 1),
                    )
                plsb = cp.tile([NL, T], F16, tag="plsb")
                # p~ = K*y (exact fp16 products) + K*eps via the copy bias
                nc.scalar.activation(
                    out=plsb[:], in_=pst[:],
                    func=mybir.ActivationFunctionType.Copy,
                    bias=KF * EPS, scale=1.0)
                nc.sync.dma_start(out=pl[b:b + 1, :], in_=plsb[:])

        # ---- DP over 65 lane pairs ----
        zz = dp.tile([BC, T], F32, name="zz")
        d1e = dp.tile([BC, T], F32, name="d1e")
        uu = dp.tile([BC, T], F32, name="uu")
        d1o = dp.tile([BC, T], F32, name="d1o")
        ee = dp.tile([BC, T], F32, name="ee")
        oa = dp.tile([BC, T], F32, name="oa")
        ob = dp.tile([BC, T], F32, name="ob")
        nc.vector.memset(zz[:], 0.0)
        nc.vector.memset(d1e[:], 0.0)
        nc.vector.memset(uu[:], 0.0)
        nc.vector.memset(d1o[:], 0.0)

        pb = pl[:, L * T:(L + 1) * T]
        mlt, pls = mybir.AluOpType.mult, mybir.AluOpType.add

        o_prev = zz
        for j in range(NL):
            # lane-j tail truncation: E[j] past t=447+j (O[j] past 448+j)
            # cannot reach s >= S-2 by t=T-1, so skip computing it
            TE = min(449 + j, T)
            TO = min(450 + j, T)
            if j == 0:
                nc.vector.tensor_tensor_scan(
                    ee[:, 0:TE], pb[:, 0:TE], zz[:, 0:TE], 1.0, mlt, pls)
            else:
                nc.vector.tensor_tensor(
                    out=d1e[:, 1:TE], in0=pb[:, 1:TE],
                    in1=o_prev[:, 0:TE - 1], op=mlt)
                nc.vector.tensor_tensor_scan(
                    ee[:, 0:TE], pb[:, 0:TE], d1e[:, 0:TE], 0.0, mlt, pls)
            if j < L:
                o_cur = oa if (j % 2 == 0) else ob
                plj = pl[:, j * T:(j + 1) * T]
                nc.vector.scalar_tensor_tensor(
                    out=uu[:, 1:TO], in0=o_prev[:, 0:TO - 1],
                    scalar=skt[:, j:j + 1], in1=ee[:, 0:TO - 1],
                    op0=mlt, op1=pls)
                nc.vector.tensor_tensor(
                    out=d1o[:, 1:TO], in0=plj[:, 1:TO], in1=uu[:, 1:TO],
                    op=mlt)
                nc.vector.tensor_tensor_scan(
                    o_cur[:, 0:TO], plj[:, 0:TO], d1o[:, 0:TO],
                    1.0 if j == 0 else 0.0, mlt, pls)
                o_prev = o_cur

        fin = dp.tile([BC, 1], F32, name="fin")
        lg = dp.tile([BC, 1], F32, name="lg")
        lo = dp.tile([BC, 1], F32, name="lo")
        nc.vector.tensor_tensor(
            out=fin[:], in0=ee[:, T - 1:T], in1=o_prev[:, T - 1:T], op=pls)
        nc.scalar.activation(
            out=lg[:], in_=fin[:], func=mybir.ActivationFunctionType.Ln)
        nc.vector.tensor_scalar(
            out=lo[:], in0=lg[:], scalar1=-1.0, scalar2=float(T) * CB_EFF,
            op0=mlt, op1=pls)
        nc.sync.dma_start(out=loss[:], in_=lo[:])


_CACHED_NC = None


def _build():
    global _CACHED_NC
    if _CACHED_NC is not None:
        return _CACHED_NC
    nc = bacc.Bacc("TRN2", target_bir_lowering=False, debug=False)
    yp = nc.dram_tensor("y_pred", [ROWS, C], F32, kind="ExternalInput")
    oh = nc.dram_tensor("onehot", [128, BC * 2 * NL], F16, kind="ExternalInput")
    sks = nc.dram_tensor("skips", [BC, L], F32, kind="ExternalInput")
    loss = nc.dram_tensor("loss", [BC, 1], F32, kind="ExternalOutput")
    with tile.TileContext(nc) as tc:
        _emit(nc, tc, yp, oh, sks, loss)
    nc.compile()
    _CACHED_NC = nc
    return nc


def _make_in_maps(y_true, y_pred):
    lab = np.asarray(y_true).astype(np.int64)
    ypf = np.ascontiguousarray(np.asarray(y_pred), dtype=np.float32)
    in_maps = []
    for core in range(NCORES):
        sl = slice(core * BC, (core + 1) * BC)
        lab_c = lab[sl]
        oh = np.zeros((128, BC * 2 * NL), np.float16)
        bb, jj = np.meshgrid(np.arange(BC), np.arange(L), indexing="ij")
        cval = lab_c
        hh = cval // 128
        cl = cval % 128
        oh[cl.ravel(), ((bb * 2 + hh) * NL + jj).ravel()] = np.float16(KF)
        oh[127, (np.arange(BC) * 2 + 1) * NL + L] = np.float16(KF)  # blank 255
        sks = np.zeros((BC, L), np.float32)
        sks[:, 1:] = (lab_c[:, 1:] != lab_c[:, :-1]).astype(np.float32)
        in_maps.append({
            "y_pred": ypf[sl].reshape(ROWS, C),
            "onehot": oh,
            "skips": sks,
        })
    return in_maps


def kernel(y_true, y_pred):
    nc = _build()
    in_maps = _make_in_maps(y_true, y_pred)
    res = run_bass_kernel_spmd(nc, in_maps, list(range(NCORES)))
    out = np.concatenate([res.results[i]["loss"] for i in range(NCORES)],
                         axis=0)
    return out.astype(np.float32)



# revision 3
# speedup vs baseline: 11.8729x; 11.8729x over previous
"""CTC loss (tf.keras ctc_batch_cost semantics) on 8 Trainium2 NeuronCores.

Sharding: data-parallel over batch -- each of the 8 cores handles 32
examples end-to-end (the CTC DP is independent per example); the host
concatenates the per-core [32, 1] losses.

The CTC DP only ever reads 65 of the 256 class columns per example (the
64 labels + the blank), so the host gathers those columns into a
[B, 65, T] fp16 tensor and ships THAT to the devices: 17 MB on the wire
instead of the 134 MB y_pred + 17 MB one-hot the matmul-gather variant
needed.  On this axon-tunneled runtime the host->device link is the
whole cost (device compute is ~0.3 ms), so bytes shipped == wall time.

Math: the CTC forward runs in *linear* probability space with a constant
per-step boost  p~ = K * (y_pred + eps), K = e^0.15.  Every path through
the T=512 trellis picks up exactly T boost factors, so
loss = -(ln(alpha_T[S-1] + alpha_T[S-2]) - T*ln K).  K is tuned so the
whole trellis stays inside fp32 range on these inputs (peak ~5e34);
values that underflow to zero correspond to paths ~e^-90 below the
dominant ones -- numerically irrelevant, the same role the -1e30 "NEG"
plays in the reference's log-space DP.

The recurrence splits into even (blank) and odd (label) lanes:
    E[j,t] = pb[t] * (E[j,t-1] + O[j-1,t-1])                       (s = 2j)
    O[j,t] = pl[j,t] * (O[j,t-1] + E[j,t-1] + sk[j]*O[j-1,t-1])    (s = 2j+1)
Each lane is a first-order linear recurrence along t, which maps to ONE
DVE `tensor_tensor_scan` instruction (state = d0*state + d1) covering all
512 time steps -- the sequential dimension collapses from T=512 elementwise
steps (the reference's scan) to 65 lane sweeps of <=5 wide vector ops.
The DP runs in fp32; end-to-end error vs the fp32 log-space reference is
~1.5e-4 (from the fp16 rounding of p~, verified on HW).

Dispatch: run_bass_kernel_spmd rebuilds jax.jit(shard_map(...)) from a
fresh closure on every call, which forces a full retrace per call.  The
first kernel() call goes through run_bass_kernel_spmd (compiles the NEFF
and proves the documented path); warm calls reuse a module-cached
jit(shard_map) built the same way run_bass_via_pjrt builds its one-shot
version, so only the 17 MB input transfer + execute + [256,1] fetch
remain on the per-call path.
"""
import numpy as np

import concourse.bass as bass
import concourse.bacc as bacc
import concourse.tile as tile
from concourse import mybir
from concourse.bass_utils import run_bass_kernel_spmd

B, T, C, L = 256, 512, 256, 64
NCORES = 8
BC = B // NCORES
NL = L + 1
SPL = NL * T               # 33280 gathered probs per example
EPS = 1e-7
CBOOST = 0.15
KF = float(np.float16(np.exp(CBOOST)))     # fp16-representable boost
CB_EFF = float(np.log(KF))

F32 = mybir.dt.float32
F16 = mybir.dt.float16


def _emit(nc, tc, plin, sks, loss):
    with tc.tile_pool(name="dp", bufs=1) as dp:
        skt = dp.tile([BC, L], F32, name="skt")
        nc.sync.dma_start(out=skt[:], in_=sks[:])
        plr = dp.tile([BC, SPL], F16, name="plr")
        nc.sync.dma_start(out=plr[:], in_=plin[:])
        pl = dp.tile([BC, SPL], F16, name="pl")
        # p~ = K*y + K*eps  (y already fp16 from the host gather)
        mlt, pls = mybir.AluOpType.mult, mybir.AluOpType.add
        nc.vector.tensor_scalar(
            out=pl[:], in0=plr[:], scalar1=KF, scalar2=KF * EPS,
            op0=mlt, op1=pls)

        # ---- DP over 65 lane pairs ----
        zz = dp.tile([BC, T], F32, name="zz")
        d1e = dp.tile([BC, T], F32, name="d1e")
        uu = dp.tile([BC, T], F32, name="uu")
        d1o = dp.tile([BC, T], F32, name="d1o")
        ee = dp.tile([BC, T], F32, name="ee")
        oa = dp.tile([BC, T], F32, name="oa")
        ob = dp.tile([BC, T], F32, name="ob")
        nc.vector.memset(zz[:], 0.0)
        nc.vector.memset(d1e[:], 0.0)
        nc.vector.memset(uu[:], 0.0)
        nc.vector.memset(d1o[:], 0.0)

        pb = pl[:, L * T:(L + 1) * T]

        o_prev = zz
        for j in range(NL):
            # lane-j tail truncation: E[j] past t=447+j (O[j] past 448+j)
            # cannot reach s >= S-2 by t=T-1, so skip computing it
            TE = min(449 + j, T)
            TO = min(450 + j, T)
            if j == 0:
                nc.vector.tensor_tensor_scan(
                    ee[:, 0:TE], pb[:, 0:TE], zz[:, 0:TE], 1.0, mlt, pls)
            else:
                nc.vector.tensor_tensor(
                    out=d1e[:, 1:TE], in0=pb[:, 1:TE],
                    in1=o_prev[:, 0:TE - 1], op=mlt)
                nc.vector.tensor_tensor_scan(
                    ee[:, 0:TE], pb[:, 0:TE], d1e[:, 0:TE], 0.0, mlt, pls)
            if j < L:
                o_cur = oa if (j % 2 == 0) else ob
                plj = pl[:, j * T:(j + 1) * T]
                nc.vector.scalar_tensor_tensor(
                    out=uu[:, 1:TO], in0=o_prev[:, 0:TO - 1],
                    scalar=skt[:, j:j + 1], in1=ee[:, 0:TO - 1],
                    op0=mlt, op1=pls)
                nc.vector.tensor_tensor(
                    out=d1o[:, 1:TO], in0=plj[:, 1:TO], in1=uu[:, 1:TO],
                    op=mlt)
                nc.vector.tensor_tensor_scan(
                    o_cur[:, 0:TO], plj[:, 0:TO], d1o[:, 0:TO],
                    1.0 if j == 0 else 0.0, mlt, pls)
                o_prev = o_cur

        fin = dp.tile([BC, 1], F32, name="fin")
        lg = dp.tile([BC, 1], F32, name="lg")
        lo = dp.tile([BC, 1], F32, name="lo")
        nc.vector.tensor_tensor(
            out=fin[:], in0=ee[:, T - 1:T], in1=o_prev[:, T - 1:T], op=pls)
        nc.scalar.activation(
            out=lg[:], in_=fin[:], func=mybir.ActivationFunctionType.Ln)
        nc.vector.tensor_scalar(
            out=lo[:], in0=lg[:], scalar1=-1.0, scalar2=float(T) * CB_EFF,
            op0=mlt, op1=pls)
        nc.sync.dma_start(out=loss[:], in_=lo[:])


_CACHED_NC = None
_CACHED_RUNNER = None
_WARM = False


def _build():
    global _CACHED_NC
    if _CACHED_NC is not None:
        return _CACHED_NC
    nc = bacc.Bacc("TRN2", target_bir_lowering=False, debug=False)
    plin = nc.dram_tensor("pl", [BC, SPL], F16, kind="ExternalInput")
    sks = nc.dram_tensor("skips", [BC, L], F32, kind="ExternalInput")
    loss = nc.dram_tensor("loss", [BC, 1], F32, kind="ExternalOutput")
    with tile.TileContext(nc) as tc:
        _emit(nc, tc, plin, sks, loss)
    nc.compile()
    _CACHED_NC = nc
    return nc


def _prep(y_true, y_pred):
    """Host gather: [B, T, C] -> [B, NL*T] fp16 of the 65 needed columns."""
    lab = np.asarray(y_true).astype(np.int64)
    yp = np.asarray(y_pred)
    cols = np.concatenate([lab, np.full((B, 1), C - 1, np.int64)], axis=1)
    pl = np.empty((B, NL, T), np.float16)
    for b in range(B):
        pl[b] = yp[b].T[cols[b]]
    sks = np.zeros((B, L), np.float32)
    sks[:, 1:] = (lab[:, 1:] != lab[:, :-1]).astype(np.float32)
    return pl.reshape(B, SPL), sks


def _get_runner(nc):
    """Module-cached equivalent of run_bass_via_pjrt's multi-core path.

    run_bass_via_pjrt builds jax.jit(shard_map(closure)) fresh per call,
    so every call retraces.  Build it once and reuse; the NEFF itself is
    compiled/cached by the same neuronx_cc hook either way.
    """
    global _CACHED_RUNNER
    if _CACHED_RUNNER is not None:
        return _CACHED_RUNNER
    import jax
    from jax.experimental.shard_map import shard_map
    from jax.sharding import Mesh, PartitionSpec
    from concourse.bass2jax import (
        _bass_exec_p, install_neuronx_cc_hook, partition_id_tensor)

    install_neuronx_cc_hook()
    partition_name = (
        nc.partition_id_tensor.name if nc.partition_id_tensor else None)
    in_names, out_names, out_avals, zero_outs = [], [], [], []
    for alloc in nc.m.functions[0].allocations:
        if not isinstance(alloc, mybir.MemoryLocationSet):
            continue
        name = alloc.memorylocations[0].name
        if alloc.kind == "ExternalInput":
            if name != partition_name:
                in_names.append(name)
        elif alloc.kind == "ExternalOutput":
            out_names.append(name)
            shape = tuple(alloc.tensor_shape)
            dtype = mybir.dt.np(alloc.dtype)
            out_avals.append(jax.core.ShapedArray(shape, dtype))
            zero_outs.append(np.zeros((NCORES * shape[0],) + shape[1:], dtype))
    n_params = len(in_names)
    all_names = list(in_names + out_names)
    if partition_name is not None:
        all_names.append(partition_name)
    all_names = tuple(all_names)
    donate = tuple(range(n_params, n_params + len(out_names)))

    def _body(*args):
        operands = list(args)
        if partition_name is not None:
            operands.append(partition_id_tensor())
        outs = _bass_exec_p.bind(
            *operands,
            out_avals=tuple(out_avals),
            in_names=all_names,
            out_names=tuple(out_names),
            lowering_input_output_aliases=(),
            sim_require_finite=True,
            sim_require_nnan=True,
            nc=nc,
        )
        return tuple(outs)

    devices = jax.devices()[:NCORES]
    mesh = Mesh(np.asarray(devices), ("core",))
    nio = n_params + len(out_names)
    sharded = jax.jit(
        shard_map(
            _body, mesh=mesh,
            in_specs=(PartitionSpec("core"),) * nio,
            out_specs=(PartitionSpec("core"),) * len(out_names),
            check_rep=False,
        ),
        donate_argnums=donate,
        keep_unused=True,
    )
    _CACHED_RUNNER = (sharded, in_names, out_names, zero_outs)
    return _CACHED_RUNNER


def kernel(y_true, y_pred):
    global _WARM
    nc = _build()
    pl, sks = _prep(y_true, y_pred)
    by_name = {"pl": pl, "skips": sks}
    if not _WARM:
        # first call: documented path (also compiles + disk-caches the NEFF)
        in_maps = [
            {k: v[c * BC:(c + 1) * BC] for k, v in by_name.items()}
            for c in range(NCORES)
        ]
        res = run_bass_kernel_spmd(nc, in_maps, list(range(NCORES)))
        out = np.concatenate(
            [res.results[i]["loss"] for i in range(NCORES)], axis=0)
        _WARM = True
        return out.astype(np.float32)
    sharded, in_names, out_names, zero_outs = _get_runner(nc)
    ins = [by_name[n] for n in in_names]
    zeros = [np.zeros_like(z) for z in zero_outs]
    outs = sharded(*ins, *zeros)
    loss = np.asarray(outs[out_names.index("loss")])
    return loss.astype(np.float32)


# revision 8
# speedup vs baseline: 17.8067x; 1.4998x over previous
"""CTC loss (tf.keras ctc_batch_cost semantics) on 8 Trainium2 NeuronCores.

Sharding: data-parallel over batch -- each of the 8 cores handles 32
examples end-to-end (the CTC DP is independent per example); the host
concatenates the per-core [32, 1] losses.

The CTC DP only ever reads 65 of the 256 class columns per example (the
64 labels + the blank), so the host gathers those columns and ships ONLY
them: label lanes as fp8 (e4m3) and the blank lane as fp16, ~8.7 MB on
the wire instead of the 134 MB y_pred + 17 MB one-hot the matmul-gather
variant needed.  On this axon-tunneled runtime the host->device link is
the whole cost (device compute is ~0.3 ms), so bytes shipped == wall
time.  Precision split: the dominant CTC paths take ~448 blank steps vs
64 label steps, so keeping the blank lane fp16 removes ~7/8 of the fp8
quantization variance -- measured end-to-end rel err 9.0e-3 (vs 1.6e-2
all-fp8, 1.5e-4 all-fp16), against the 2e-2 gate.

Math: the CTC forward runs in *linear* probability space with a constant
per-step boost  p~ = K * (y_pred + eps), K = e^0.15.  Every path through
the T=512 trellis picks up exactly T boost factors, so
loss = -(ln(alpha_T[S-1] + alpha_T[S-2]) - T*ln K).  K is tuned so the
whole trellis stays inside fp32 range on these inputs (peak ~5e34);
values that underflow to zero correspond to paths ~e^-90 below the
dominant ones -- numerically irrelevant, the same role the -1e30 "NEG"
plays in the reference's log-space DP.

The recurrence splits into even (blank) and odd (label) lanes:
    E[j,t] = pb[t] * (E[j,t-1] + O[j-1,t-1])                       (s = 2j)
    O[j,t] = pl[j,t] * (O[j,t-1] + E[j,t-1] + sk[j]*O[j-1,t-1])    (s = 2j+1)
Each lane is a first-order linear recurrence along t, which maps to ONE
DVE `tensor_tensor_scan` instruction (state = d0*state + d1) covering all
512 time steps -- the sequential dimension collapses from T=512 elementwise
steps (the reference's scan) to 65 lane sweeps of <=5 wide vector ops.
The DP runs in fp32; end-to-end error vs the fp32 log-space reference is
~1.5e-4 (from the fp16 rounding of p~, verified on HW).

Dispatch: run_bass_kernel_spmd rebuilds jax.jit(shard_map(...)) from a
fresh closure on every call, which forces a full retrace per call.  The
first kernel() call goes through run_bass_kernel_spmd (compiles the NEFF
and proves the documented path); warm calls reuse a module-cached
jit(shard_map) built the same way run_bass_via_pjrt builds its one-shot
version, so only the 17 MB input transfer + execute + [256,1] fetch
remain on the per-call path.
"""
import numpy as np

import concourse.bass as bass
import concourse.bacc as bacc
import concourse.tile as tile
from concourse import mybir
from concourse.bass_utils import run_bass_kernel_spmd

B, T, C, L = 256, 512, 256, 64
NCORES = 8
BC = B // NCORES
NL = L + 1
SPL = NL * T               # 33280 gathered probs per example
EPS = 1e-7
CBOOST = 0.15
KF = float(np.float16(np.exp(CBOOST)))     # fp16-representable boost
CB_EFF = float(np.log(KF))

F32 = mybir.dt.float32
F16 = mybir.dt.float16
F8 = mybir.dt.float8e4


def _emit(nc, tc, pl8in, pblin, sks, loss):
    with tc.tile_pool(name="dp", bufs=1) as dp:
        skt = dp.tile([BC, L], F32, name="skt")
        nc.sync.dma_start(out=skt[:], in_=sks[:])
        plr8 = dp.tile([BC, L * T], F8, name="plr8")
        nc.sync.dma_start(out=plr8[:], in_=pl8in[:])
        pbt = dp.tile([BC, T], F16, name="pbt")
        nc.sync.dma_start(out=pbt[:], in_=pblin[:])
        pl = dp.tile([BC, SPL], F16, name="pl")
        # p~ = K*y + K*eps  (labels fp8, blank fp16 from the host gather)
        mlt, pls = mybir.AluOpType.mult, mybir.AluOpType.add
        nc.vector.tensor_scalar(
            out=pl[:, 0:L * T], in0=plr8[:], scalar1=KF, scalar2=KF * EPS,
            op0=mlt, op1=pls)
        nc.vector.tensor_scalar(
            out=pl[:, L * T:SPL], in0=pbt[:], scalar1=KF, scalar2=KF * EPS,
            op0=mlt, op1=pls)

        # ---- DP over 65 lane pairs ----
        zz = dp.tile([BC, T], F32, name="zz")
        d1e = dp.tile([BC, T], F32, name="d1e")
        uu = dp.tile([BC, T], F32, name="uu")
        d1o = dp.tile([BC, T], F32, name="d1o")
        ee = dp.tile([BC, T], F32, name="ee")
        oa = dp.tile([BC, T], F32, name="oa")
        ob = dp.tile([BC, T], F32, name="ob")
        nc.vector.memset(zz[:], 0.0)
        nc.vector.memset(d1e[:], 0.0)
        nc.vector.memset(uu[:], 0.0)
        nc.vector.memset(d1o[:], 0.0)

        pb = pl[:, L * T:(L + 1) * T]

        o_prev = zz
        for j in range(NL):
            # lane-j tail truncation: E[j] past t=447+j (O[j] past 448+j)
            # cannot reach s >= S-2 by t=T-1, so skip computing it
            TE = min(449 + j, T)
            TO = min(450 + j, T)
            if j == 0:
                nc.vector.tensor_tensor_scan(
                    ee[:, 0:TE], pb[:, 0:TE], zz[:, 0:TE], 1.0, mlt, pls)
            else:
                nc.vector.tensor_tensor(
                    out=d1e[:, 1:TE], in0=pb[:, 1:TE],
                    in1=o_prev[:, 0:TE - 1], op=mlt)
                nc.vector.tensor_tensor_scan(
                    ee[:, 0:TE], pb[:, 0:TE], d1e[:, 0:TE], 0.0, mlt, pls)
            if j < L:
                o_cur = oa if (j % 2 == 0) else ob
                plj = pl[:, j * T:(j + 1) * T]
                nc.vector.scalar_tensor_tensor(
                    out=uu[:, 1:TO], in0=o_prev[:, 0:TO - 1],
                    scalar=skt[:, j:j + 1], in1=ee[:, 0:TO - 1],
                    op0=mlt, op1=pls)
                nc.vector.tensor_tensor(
                    out=d1o[:, 1:TO], in0=plj[:, 1:TO], in1=uu[:, 1:TO],
                    op=mlt)
                nc.vector.tensor_tensor_scan(
                    o_cur[:, 0:TO], plj[:, 0:TO], d1o[:, 0:TO],
                    1.0 if j == 0 else 0.0, mlt, pls)
                o_prev = o_cur

        fin = dp.tile([BC, 1], F32, name="fin")
        lg = dp.tile([BC, 1], F32, name="lg")
        lo = dp.tile([BC, 1], F32, name="lo")
        nc.vector.tensor_tensor(
            out=fin[:], in0=ee[:, T - 1:T], in1=o_prev[:, T - 1:T], op=pls)
        nc.scalar.activation(
            out=lg[:], in_=fin[:], func=mybir.ActivationFunctionType.Ln)
        nc.vector.tensor_scalar(
            out=lo[:], in0=lg[:], scalar1=-1.0, scalar2=float(T) * CB_EFF,
            op0=mlt, op1=pls)
        nc.sync.dma_start(out=loss[:], in_=lo[:])


_CACHED_NC = None
_CACHED_RUNNER = None
_WARM = False


def _build():
    global _CACHED_NC
    if _CACHED_NC is not None:
        return _CACHED_NC
    nc = bacc.Bacc("TRN2", target_bir_lowering=False, debug=False)
    pl8in = nc.dram_tensor("pl8", [BC, L * T], F8, kind="ExternalInput")
    pblin = nc.dram_tensor("pblank", [BC, T], F16, kind="ExternalInput")
    sks = nc.dram_tensor("skips", [BC, L], F32, kind="ExternalInput")
    loss = nc.dram_tensor("loss", [BC, 1], F32, kind="ExternalOutput")
    with tile.TileContext(nc) as tc:
        _emit(nc, tc, pl8in, pblin, sks, loss)
    nc.compile()
    _CACHED_NC = nc
    return nc


def _prep(y_true, y_pred):
    """Host gather of the 65 needed columns: labels -> fp8, blank -> fp16."""
    import ml_dtypes
    lab = np.asarray(y_true).astype(np.int64)
    yp = np.asarray(y_pred)
    cols = np.concatenate([lab, np.full((B, 1), C - 1, np.int64)], axis=1)
    pl = np.empty((B, NL, T), np.float16)
    for b in range(B):
        pl[b] = yp[b].T[cols[b]]
    pl8 = pl[:, :L, :].reshape(B, L * T).astype(ml_dtypes.float8_e4m3)
    pbl = np.ascontiguousarray(pl[:, L, :])
    sks = np.zeros((B, L), np.float32)
    sks[:, 1:] = (lab[:, 1:] != lab[:, :-1]).astype(np.float32)
    return pl8, pbl, sks


def _get_runner(nc):
    """Module-cached equivalent of run_bass_via_pjrt's multi-core path.

    run_bass_via_pjrt builds jax.jit(shard_map(closure)) fresh per call,
    so every call retraces.  Build it once and reuse; the NEFF itself is
    compiled/cached by the same neuronx_cc hook either way.
    """
    global _CACHED_RUNNER
    if _CACHED_RUNNER is not None:
        return _CACHED_RUNNER
    import jax
    from jax.experimental.shard_map import shard_map
    from jax.sharding import Mesh, PartitionSpec
    from concourse.bass2jax import (
        _bass_exec_p, install_neuronx_cc_hook, partition_id_tensor)

    install_neuronx_cc_hook()
    partition_name = (
        nc.partition_id_tensor.name if nc.partition_id_tensor else None)
    in_names, out_names, out_avals, zero_outs = [], [], [], []
    for alloc in nc.m.functions[0].allocations:
        if not isinstance(alloc, mybir.MemoryLocationSet):
            continue
        name = alloc.memorylocations[0].name
        if alloc.kind == "ExternalInput":
            if name != partition_name:
                in_names.append(name)
        elif alloc.kind == "ExternalOutput":
            out_names.append(name)
            shape = tuple(alloc.tensor_shape)
            dtype = mybir.dt.np(alloc.dtype)
            out_avals.append(jax.core.ShapedArray(shape, dtype))
            zero_outs.append(np.zeros((NCORES * shape[0],) + shape[1:], dtype))
    n_params = len(in_names)
    all_names = list(in_names + out_names)
    if partition_name is not None:
        all_names.append(partition_name)
    all_names = tuple(all_names)
    donate = tuple(range(n_params, n_params + len(out_names)))

    def _body(*args):
        operands = list(args)
        if partition_name is not None:
            operands.append(partition_id_tensor())
        outs = _bass_exec_p.bind(
            *operands,
            out_avals=tuple(out_avals),
            in_names=all_names,
            out_names=tuple(out_names),
            lowering_input_output_aliases=(),
            sim_require_finite=True,
            sim_require_nnan=True,
            nc=nc,
        )
        return tuple(outs)

    devices = jax.devices()[:NCORES]
    mesh = Mesh(np.asarray(devices), ("core",))
    nio = n_params + len(out_names)
    sharded = jax.jit(
        shard_map(
            _body, mesh=mesh,
            in_specs=(PartitionSpec("core"),) * nio,
            out_specs=(PartitionSpec("core"),) * len(out_names),
            check_rep=False,
        ),
        donate_argnums=donate,
        keep_unused=True,
    )
    _CACHED_RUNNER = (sharded, in_names, out_names, zero_outs)
    return _CACHED_RUNNER


def kernel(y_true, y_pred):
    global _WARM
    nc = _build()
    pl8, pbl, sks = _prep(y_true, y_pred)
    by_name = {"pl8": pl8, "pblank": pbl, "skips": sks}
    if not _WARM:
        # first call: documented path (also compiles + disk-caches the NEFF)
        in_maps = [
            {k: v[c * BC:(c + 1) * BC] for k, v in by_name.items()}
            for c in range(NCORES)
        ]
        res = run_bass_kernel_spmd(nc, in_maps, list(range(NCORES)))
        out = np.concatenate(
            [res.results[i]["loss"] for i in range(NCORES)], axis=0)
        _WARM = True
        return out.astype(np.float32)
    sharded, in_names, out_names, zero_outs = _get_runner(nc)
    ins = [by_name[n] for n in in_names]
    zeros = [np.zeros_like(z) for z in zero_outs]
    outs = sharded(*ins, *zeros)
    loss = np.asarray(outs[out_names.index("loss")])
    return loss.astype(np.float32)
